# revision 23
# baseline (speedup 1.0000x reference)
"""Row-normalize kernel for nn_EstimateAdj (N=8192) on 8 trn2 NeuronCores.

Math (per reference):
    mx     = estimated_adj * ori + I
    rowsum = mx.sum(axis=1)
    out    = (1/rowsum)[:, None] * mx

Sharding: 1D row partition across 8 cores (1024 rows each). Row-sum,
reciprocal and row-scale are row-local, so the device program is uniform
across cores. The identity matrix is handled without any core-dependent
addressing:
  - its contribution to rowsum is the reduction's initial value (1.0)
  - its contribution to the output (out[i,i] += r_inv[i]) is an O(N)
    host-side fix-up using the r_inv values computed on device.

Per core: 8 row-tiles of [128, 8192] f32. Per tile:
  load est/ori (SP HWDGE ring) -> DVE scalar_tensor_tensor (mx = est*ori
  fused with rowsum accumulation) -> +1.0, reciprocal (DVE) -> ScalarE
  copy-with-per-row-scale (out = mx * r_inv) -> store (ACT HWDGE ring).
Loads and stores live on different HWDGE rings so a store's compute-wait
never stalls load issue. Memory-bound: 96 MB HBM traffic per core
(~268 us roofline at ~358 GB/s; measured ~300 us steady-state).
"""

import numpy as np

import concourse.bacc as bacc
import concourse.bass as bass
import concourse.tile as tile
from concourse import mybir
from concourse.bass_utils import run_bass_kernel_spmd

N = 8192
N_CORES = 8
ROWS = N // N_CORES  # rows per core
P = 128              # SBUF partitions
TILES = ROWS // P    # row-tiles per core

_NC_CACHE: dict = {}


def _build_nc(
    repeats: int = 1,
    ori_engine: str = "sync",
    store_engine: str = "scalar",
    chunk: int = N,
    est_bufs: int = 3,
    ori_bufs: int = 2,
    tail_chunks: int = 1,
) -> bass.Bass:
    """Build the per-core program. repeats>1 wraps the whole body in a
    hardware loop that redoes identical work — used only for timing.
    ori_engine: which queue issues the ori loads ('sync'|'gpsimd').
    chunk: column-chunk width for the load/mul stage (divides N).
    tail_chunks: column chunking applied ONLY to the last tile's pipeline
    to compress the end-of-kernel serial tail (load->mul->scale->store)."""
    nc = bacc.Bacc(None)
    est = nc.dram_tensor("est", [ROWS, N], mybir.dt.float32, kind="ExternalInput")
    ori = nc.dram_tensor("ori", [ROWS, N], mybir.dt.float32, kind="ExternalInput")
    out = nc.dram_tensor("out", [ROWS, N], mybir.dt.float32, kind="ExternalOutput")
    # [P, TILES]: rinv[p, t] = 1/rowsum of local row t*P+p (host transposes)
    rinv = nc.dram_tensor("rinv", [P, TILES], mybir.dt.float32, kind="ExternalOutput")

    from contextlib import ExitStack, nullcontext

    n_chunks = N // chunk
    ori_eng = {"sync": nc.sync, "gpsimd": nc.gpsimd}[ori_engine]
    st_eng = {"scalar": nc.scalar, "gpsimd": nc.gpsimd, "sync": nc.sync}[store_engine]

    with tile.TileContext(nc) as tc, ExitStack() as ctx:
        est_pool = ctx.enter_context(tc.tile_pool(name="est_pool", bufs=est_bufs))
        ori_pool = ctx.enter_context(tc.tile_pool(name="ori_pool", bufs=ori_bufs))
        small = ctx.enter_context(tc.tile_pool(name="small", bufs=4))
        singles = ctx.enter_context(tc.tile_pool(name="singles", bufs=1))
        with tc.For_i(0, repeats, 1) if repeats > 1 else nullcontext():
            # r_inv for all tiles, written column t per tile, one store at end
            rinv_all = singles.tile([P, TILES], mybir.dt.float32)
            for t in range(TILES):
                r0 = t * P
                nch = tail_chunks if t == TILES - 1 else n_chunks
                cw = N // nch
                # full-width mx tile; chunk loads/compute fill it piecewise
                est_t = est_pool.tile([P, N], mybir.dt.float32)
                sums = small.tile([P, nch], mybir.dt.float32, tag="sums")
                last_ori = None
                for c in range(nch):
                    c0 = c * cw
                    ori_c = ori_pool.tile([P, cw], mybir.dt.float32, tag="ori_c")
                    last_ori = ori_c
                    # loads on SP (+optionally SWDGE) rings — stores go via ACT
                    # so a store's compute-wait never blocks load issue
                    nc.sync.dma_start(
                        out=est_t[:, c0 : c0 + cw],
                        in_=est[r0 : r0 + P, c0 : c0 + cw],
                    )
                    ori_eng.dma_start(
                        out=ori_c[:, 0:cw], in_=ori[r0 : r0 + P, c0 : c0 + cw]
                    )
                    # mx_chunk = est*ori in-place into est_t; sums[c]=rowsum
                    nc.vector.scalar_tensor_tensor(
                        out=est_t[:, c0 : c0 + cw],
                        in0=est_t[:, c0 : c0 + cw],
                        scalar=1.0,
                        in1=ori_c[:, 0:cw],
                        op0=mybir.AluOpType.mult,
                        op1=mybir.AluOpType.mult,
                        accum_out=sums[:, c : c + 1],
                    )
                rowsum = small.tile([P, 1], mybir.dt.float32, tag="rowsum")
                if nch > 1:
                    nc.vector.reduce_sum(
                        rowsum[:], sums[:, 0:nch], axis=mybir.AxisListType.X
                    )
                    # +1.0 accounts for the identity's diagonal in this row
                    nc.vector.tensor_scalar_add(rowsum[:], rowsum[:], 1.0)
                else:
                    nc.vector.tensor_scalar_add(rowsum[:], sums[:, 0:1], 1.0)
                nc.vector.reciprocal(out=rinv_all[:, t : t + 1], in_=rowsum[:])
                # out = mx * r_inv on ScalarE (per-partition scale), store via ACT
                if nch == 1:
                    # reuse the consumed ori tile as the out buffer (saves SBUF)
                    nc.scalar.mul(
                        out=last_ori[:], in_=est_t[:], mul=rinv_all[:, t : t + 1]
                    )
                    st_eng.dma_start(out=out[r0 : r0 + P, :], in_=last_ori[:])
                else:
                    for c in range(nch):
                        c0 = c * cw
                        out_c = ori_pool.tile([P, cw], mybir.dt.float32, tag="out_c")
                        nc.scalar.mul(
                            out=out_c[:, 0:cw],
                            in_=est_t[:, c0 : c0 + cw],
                            mul=rinv_all[:, t : t + 1],
                        )
                        st_eng.dma_start(
                            out=out[r0 : r0 + P, c0 : c0 + cw], in_=out_c[:, 0:cw]
                        )
            st_eng.dma_start(out=rinv[:, :], in_=rinv_all[:])
    nc.finalize()
    return nc


def _get_nc(repeats: int = 1) -> bass.Bass:
    if repeats not in _NC_CACHE:
        _NC_CACHE[repeats] = _build_nc(repeats)
    return _NC_CACHE[repeats]


def run_sharded(estimated_adj: np.ndarray, ori: np.ndarray, repeats: int = 1, **run_kwargs):
    """Shard inputs, run the SPMD kernel on 8 cores, return BassKernelResults."""
    est = np.ascontiguousarray(np.asarray(estimated_adj, dtype=np.float32))
    orig = np.ascontiguousarray(np.asarray(ori, dtype=np.float32))
    in_maps = [
        {
            "est": est[c * ROWS : (c + 1) * ROWS],
            "ori": orig[c * ROWS : (c + 1) * ROWS],
        }
        for c in range(N_CORES)
    ]
    return run_bass_kernel_spmd(_get_nc(repeats), in_maps, list(range(N_CORES)), **run_kwargs)


def assemble(results) -> np.ndarray:
    """Gather per-core outputs into the full [N, N] result (with diag fix)."""
    out = np.concatenate([r["out"] for r in results], axis=0)
    # rinv[p, t] = 1/rowsum of local row t*128+p -> transpose to row order
    rinv = np.concatenate([np.asarray(r["rinv"]).T.reshape(-1) for r in results])
    idx = np.arange(N)
    out[idx, idx] += rinv
    return out


def _plausible(out: np.ndarray) -> bool:
    # out is row-normalized: every row sums to 1 (or 0 for the inf->0 rows,
    # which cannot occur for these inputs). A cheap invariant that catches
    # the occasional post-wedge device corruption (unscaled rows sum to ~2049).
    rs = out.sum(axis=1, dtype=np.float64)
    return bool(np.all(np.abs(rs - 1.0) < 1e-2))


def kernel(estimated_adj: np.ndarray, ori: np.ndarray) -> np.ndarray:
    import time

    out = None
    for attempt in range(3):
        try:
            out = assemble(run_sharded(estimated_adj, ori).results)
        except Exception:
            # the axon-proxied device occasionally reports "unrecoverable"
            # right after another session closed; a delayed retry recovers it
            if attempt == 2:
                raise
            time.sleep(20)
            continue
        if _plausible(out):
            break
        time.sleep(10)
    return out
